# revision 22
# baseline (speedup 1.0000x reference)
"""Trainium2 Bass kernel for nn_MetaRouter (dense_transformer).

Contract: kernel(**inputs) takes FULL unsharded inputs (as produced by
reference.setup_inputs()) and returns the FULL [B, D] logits, matching
reference.reference(**inputs).

Strategy:
  - Data-parallel over batch: B=16 split as 2 rows per core x 8 cores.
    All parameters replicated. No collectives.
  - Host side: tokens with attention_mask==0 get softmax weight exactly 0
    for every query, so each row is compacted to its unmasked tokens
    (padded to a multiple of 128; pad slots get a -1e9 score bias).
    ts is pre-cast to bf16 and pre-transposed into [tile, 128 feat, tok]
    chunks so the chip never transposes it. The 17 attention queries are
    folded into the projection weight matrix as extra columns:
        Q_hat = W @ q - w_bar * colsum(q)   (w_bar = row-mean of W)
    which makes raw_score[s,q] = ts_s . Q_hat[:,q] = v_s.q - mu_s*sum(q),
    i.e. the LN mean-correction is pre-applied; only the per-token rstd
    scaling remains. So scores cost 17 extra matmul columns, not a
    separate pass, and x^T never needs to exist on chip.
  - Softmax denominators and the LN mean shift both cancel inside the
    downstream LayerNorms (LN is invariant to positive scaling and
    uniform shifts), so the context sums use unnormalized exp weights
    against the raw (pre-LN) projections, with the per-token rstd folded
    into the exp weights. No reduce_max, no reciprocal, no renorm.
  - Per tile of 128 tokens: 64 matmuls (32 k-chunks x 2 PSUM splits of
    265+264 columns; one PSUM bank each, LDWEIGHTS fully hidden), then
    DVE does bn_stats/bn_aggr + a bitcast-Newton rsqrt (keeps the ACT
    table pinned on Exp), ACT does exp(rstd*raw + padbias) in a single
    fused instruction, and the per-row context accumulates incrementally
    in PSUM via one [128,17]x[128,512] matmul interleaved into the
    projection stream.
  - Tail per row: LN (DVE rsqrt), PE transposes for the FFN operands,
    gelu-FFN with all weights pre-chunked, output head folded with the
    temperature on the host.
"""

import os

import numpy as np
import ml_dtypes

import concourse.bass as bass
import concourse.bacc as bacc
import concourse.tile as tile
from concourse import mybir
from concourse.masks import make_identity

P = 128
H = 512
TOKD = 4096
KC = TOKD // P    # 32 k-chunks of the projection contraction
NQ = 17           # 1 global + 16 domain queries
WTOT = H + NQ     # 529 projection output columns
SPL = 256         # W column split; region B holds W[256:] + scores
D = 16
B = 16
S = 2048
N_CORES = 8
B_LOCAL = B // N_CORES
EPS = 1e-5
F32 = mybir.dt.float32
I32 = mybir.dt.int32
BF16 = mybir.dt.bfloat16
MAGIC = 0x5F3759DF


def build_nc(S_c: int, b_out_s: float, skip=frozenset()):
    """Build the per-core Bass program for padded/compacted seq length S_c."""
    assert S_c % P == 0
    NT = S_c // P          # token tiles per batch row
    TT = B_LOCAL * NT      # token tiles per core

    nc = bacc.Bacc("TRN2", target_bir_lowering=False, num_swdge_queues=2)

    ts = nc.declare_dram_parameter("ts", [TT * P, TOKD], BF16, isOutput=False)
    pb = nc.declare_dram_parameter("pb", [P, TT], F32, isOutput=False)
    wp = nc.declare_dram_parameter("wp", [P, KC * WTOT], BF16, isOutput=False)
    bprow = nc.declare_dram_parameter("bprow", [1, WTOT], BF16, isOutput=False)
    tg = nc.declare_dram_parameter("tg", [1, H], F32, isOutput=False)
    tb = nc.declare_dram_parameter("tb", [1, H], F32, isOutput=False)
    cg = nc.declare_dram_parameter("cg", [NQ, H], F32, isOutput=False)
    cb = nc.declare_dram_parameter("cb", [NQ, H], F32, isOutput=False)
    fg = nc.declare_dram_parameter("fg", [1, H], F32, isOutput=False)
    fb = nc.declare_dram_parameter("fb", [1, H], F32, isOutput=False)
    w1 = nc.declare_dram_parameter("w1", [P, 8 * H], BF16, isOutput=False)
    bf1 = nc.declare_dram_parameter("bf1", [1, H], BF16, isOutput=False)
    w2 = nc.declare_dram_parameter("w2", [P, 4 * H], BF16, isOutput=False)
    bf2 = nc.declare_dram_parameter("bf2", [1, H], BF16, isOutput=False)
    wo = nc.declare_dram_parameter("wo", [1, H], F32, isOutput=False)
    out = nc.declare_dram_parameter("out", [D, B_LOCAL], F32, isOutput=True)

    with tile.TileContext(nc) as tc:
        _emit(tc, nc, NT, TT, b_out_s, skip,
              ts=ts, pb=pb, wp=wp, bprow=bprow, tg=tg, tb=tb, cg=cg, cb=cb,
              fg=fg, fb=fb, w1=w1, bf1=bf1, w2=w2, bf2=bf2, wo=wo, out=out)
    nc.compile()
    return nc


def _emit(tc, nc, NT, TT, b_out_s, skip, *, ts, pb, wp, bprow, tg, tb, cg, cb,
          fg, fb, w1, bf1, w2, bf2, wo, out):
    from contextlib import ExitStack
    ctx = ExitStack()
    with ctx:
        const = ctx.enter_context(tc.tile_pool(name="const", bufs=1))
        tsp = ctx.enter_context(tc.tile_pool(name="tsp", bufs=6))
        xp = ctx.enter_context(tc.tile_pool(name="xp", bufs=1))
        lnp = ctx.enter_context(tc.tile_pool(name="lnp", bufs=2))
        p2 = ctx.enter_context(tc.tile_pool(name="p2", bufs=2))
        psx = ctx.enter_context(tc.tile_pool(name="psx", bufs=2, space="PSUM"))
        ctxp = ctx.enter_context(tc.tile_pool(name="ctxp", bufs=1, space="PSUM"))
        pst = ctx.enter_context(tc.tile_pool(name="pst", bufs=1, space="PSUM"))
        ffnp = ctx.enter_context(tc.tile_pool(name="ffnp", bufs=2, space="PSUM"))

        # ---- weights / first ts tiles lead the DMA rings ----
        # W' goes chunk-by-chunk round-robin over three HWDGE rings so the
        # k-loop of tile 0 never waits on a monolithic transfer; padbias
        # leads the vector ring (needed by post(0)).
        w_sb = const.tile([P, KC, WTOT], BF16)
        _wp = wp.ap().rearrange("p (c w) -> p c w", w=WTOT)
        pb_sb = const.tile([P, TT], F32)
        nc.sync.dma_start(out=pb_sb, in_=pb.ap())
        # Startup is DMA-bound on W' (4.3MB) + the first two ts tiles: spread
        # W' as 2-chunk blocks round-robin over all three DMA paths, with the
        # first tiles' halves slotted after each path's first block. Tiles 0/1
        # are then consumed k-block-major so the PE follows the arrival order.
        prefetched = {}

        def dma_tile(t, eng=None):
            if eng is None:
                eng = (nc.gpsimd, nc.sync, nc.scalar)[t % 3]
            tt = tsp.tile([P, KC * P], BF16, tag="ts")
            eng.dma_start(out=tt, in_=ts.ap()[t * P:(t + 1) * P, :])
            prefetched[t] = tt

        # Ring DMAs hit ~280GB/s only with large contiguous per-partition
        # lines, so W' goes in four 8-chunk blocks (8.5KB lines) split across
        # both rings while ts0 leads the other ring and SWDGE carries ts1/ts2.
        dma_tile(0, nc.scalar)
        nc.sync.dma_start(out=w_sb[:, 0:8, :], in_=_wp[:, 0:8, :])
        dma_tile(1, nc.gpsimd)
        dma_tile(2, nc.gpsimd)
        nc.sync.dma_start(out=w_sb[:, 8:16, :], in_=_wp[:, 8:16, :])
        nc.scalar.dma_start(out=w_sb[:, 16:24, :], in_=_wp[:, 16:24, :])
        nc.scalar.dma_start(out=w_sb[:, 24:32, :], in_=_wp[:, 24:32, :])
        dma_tile(3, nc.sync)
        dma_tile(4, nc.scalar)
        dma_tile(5, nc.gpsimd)
        # ---- constants ----
        def bcast(dram, parts, dt=F32):
            t = const.tile([parts, H], dt, tag=f"c_{dram.name}")
            a = dram.ap()
            nc.sync.dma_start(
                out=t, in_=bass.AP(tensor=a.tensor, offset=a.offset,
                                   ap=[[0, parts]] + list(a.ap[1:])))
            return t

        w1_sb = const.tile([P, 8, H], BF16)
        nc.sync.dma_start(out=w1_sb, in_=w1.ap().rearrange("p (c h) -> p c h", h=H))
        w2_sb = const.tile([P, 4, H], BF16)
        nc.sync.dma_start(out=w2_sb, in_=w2.ap().rearrange("p (c h) -> p c h", h=H))
        wo_sb = bcast(wo, D)
        if "tln" not in skip:
            tg_sb = bcast(tg, P)
            tb_sb = bcast(tb, P)
        if "gcln" not in skip:
            cg_sb = const.tile([NQ, H], F32)
            nc.sync.dma_start(out=cg_sb, in_=cg.ap())
            cb_sb = const.tile([NQ, H], F32)
            nc.sync.dma_start(out=cb_sb, in_=cb.ap())
        if "fln" not in skip:
            fg_sb = bcast(fg, D)
            fb_sb = bcast(fb, D)
        if "bf1" not in skip:
            bf1_sb = const.tile([1, H], BF16)
            nc.sync.dma_start(out=bf1_sb, in_=bf1.ap())
        if "bf2" not in skip:
            bf2_sb = const.tile([1, H], BF16)
            nc.sync.dma_start(out=bf2_sb, in_=bf2.ap())
        if "bp" not in skip:
            bprow_sb = const.tile([1, WTOT], BF16)
            nc.sync.dma_start(out=bprow_sb, in_=bprow.ap())

        ones_row = const.tile([1, P], BF16)
        nc.vector.memset(ones_row, 1.0)
        ones_col = const.tile([P, D], BF16)
        nc.vector.memset(ones_col, 1.0)
        id17 = const.tile([NQ, NQ], BF16)
        make_identity(nc, id17)
        id16 = const.tile([D, D], BF16)
        make_identity(nc, id16)
        magic = const.tile([P, 1], I32)
        nc.vector.memset(magic, MAGIC)

        # x (raw projection, bf16) + unnormalized-attn weights, SBUF-resident
        x_sb = xp.tile([P, TT, H], BF16)
        pexpT = xp.tile([P, TT, NQ], BF16)
        logit_sb = xp.tile([D, B_LOCAL], F32)

        def rsqrt(ve, parts, tag, iters=1):
            """y ~= (ve)^-0.5 via bitcast seed + Newton steps (DVE only)."""
            y = lnp.tile([parts, 1], F32, tag=f"y_{tag}")
            sh = lnp.tile([parts, 1], I32, tag=f"sh_{tag}")
            nc.vector.tensor_scalar(out=sh, in0=ve.bitcast(I32), scalar1=1,
                                    scalar2=None,
                                    op0=mybir.AluOpType.arith_shift_right)
            nc.vector.tensor_tensor(out=y.bitcast(I32), in0=magic[:parts],
                                    in1=sh, op=mybir.AluOpType.subtract)
            t1 = lnp.tile([parts, 1], F32, tag=f"t1_{tag}")
            hh = lnp.tile([parts, 1], F32, tag=f"h_{tag}")
            for _ in range(iters):
                nc.vector.tensor_mul(out=t1, in0=y, in1=y)
                nc.vector.tensor_mul(out=t1, in0=t1, in1=ve)
                nc.vector.tensor_scalar(out=hh, in0=t1, scalar1=-0.5,
                                        scalar2=1.5, op0=mybir.AluOpType.mult,
                                        op1=mybir.AluOpType.add)
                nc.vector.tensor_mul(out=y, in0=y, in1=hh)
            return y

        psums = {}

        def proj_alloc(t):
            tsT = prefetched.pop(t).rearrange("p (c s) -> p c s", s=P)
            px = psx.tile([P, 2 * H], F32, tag="px", name="px")
            psums[t] = px
            return tsT, px

        def proj_chunk(tsT, px, k, t):
            first, last = k == 0, k == KC - 1 and "bp" in skip
            nc.tensor.matmul(px[:, 0:SPL], lhsT=tsT[:, k, :],
                             rhs=w_sb[:, k, 0:SPL], start=first, stop=last,
                             skip_group_check=True)
            nc.tensor.matmul(px[:, H:H + WTOT - SPL], lhsT=tsT[:, k, :],
                             rhs=w_sb[:, k, SPL:], start=first, stop=last,
                             skip_group_check=True)

        def proj_bias(px):
            if "bp" not in skip:
                nc.tensor.matmul(px[:, 0:SPL], lhsT=ones_row,
                                 rhs=bprow_sb[:, 0:SPL], start=False,
                                 stop=True, skip_group_check=True)
                nc.tensor.matmul(px[:, H:H + WTOT - SPL], lhsT=ones_row,
                                 rhs=bprow_sb[:, SPL:], start=False,
                                 stop=True, skip_group_check=True)

        def proj(t):
            tsT, px = proj_alloc(t)
            for k in range(KC):
                proj_chunk(tsT, px, k, t)
            proj_bias(px)

        def post(t):
            """Stats + x store + exp-weights for tile t (DVE/ACT work)."""
            px = psums.pop(t)
            vreg = px.rearrange("p (r x) -> p r x", x=H)[:, :, 0:SPL]
            stats = lnp.tile([P, 12], F32, tag="stats")
            nc.vector.bn_stats(out=stats[:, 0:6], in_=vreg[:, 0, :])
            nc.vector.bn_stats(out=stats[:, 6:12], in_=vreg[:, 1, :])
            mv = lnp.tile([P, 2], F32, tag="mv")
            nc.vector.bn_aggr(out=mv, in_=stats)
            ve = lnp.tile([P, 1], F32, tag="ve")
            nc.vector.tensor_scalar_add(out=ve, in0=mv[:, 1:2], scalar1=EPS)
            rstd = rsqrt(ve, P, "p1")
            # attn weights first: they gate the trailing ctx matmul
            nc.scalar.activation(out=pexpT[:, t, :], in_=px[:, H + H - SPL:H + WTOT - SPL],
                                 func=mybir.ActivationFunctionType.Exp,
                                 bias=pb_sb[:, t:t + 1], scale=rstd)
            xv = x_sb[:, t, :].rearrange("p (r x) -> p r x", x=SPL)
            if "tln" in skip:
                nc.vector.tensor_scalar_mul(out=pexpT[:, t, :],
                                            in0=pexpT[:, t, :], scalar1=rstd)
                # store raw v; rstd folds into the attn weights, mu cancels
                nc.vector.tensor_copy(out=xv, in_=vreg)
            else:
                xa = lnp.tile([P, H], F32, tag="xa")
                nc.vector.tensor_scalar(
                    out=xa.rearrange("p (r x) -> p r x", x=SPL), in0=vreg,
                    scalar1=mv[:, 0:1], scalar2=rstd,
                    op0=mybir.AluOpType.subtract, op1=mybir.AluOpType.mult)
                xg = lnp.tile([P, H], F32, tag="xg")
                nc.vector.tensor_mul(out=xg, in0=xa, in1=tg_sb)
                nc.vector.tensor_add(out=x_sb[:, t, :], in0=xg, in1=tb_sb)

        ctx_ps = {}

        def ctx_mm(t):
            b, i = divmod(t, NT)
            if i == 0:
                ctx_ps[b] = ctxp.tile([NQ, H], F32, tag="ctx", name="ctx")
            nc.tensor.matmul(ctx_ps[b], lhsT=pexpT[:, t, :], rhs=x_sb[:, t, :],
                             start=(i == 0), stop=(i == NT - 1))

        def row_ctx(b, warm=0):
            """Context LN + transposes + fused operand build for row b."""
            cps = ctx_ps.pop(b)
            stats = p2.tile([NQ, 6], F32, tag="stats2")
            nc.vector.bn_stats(out=stats, in_=cps)
            mv = p2.tile([NQ, 2], F32, tag="mv2")
            nc.vector.bn_aggr(out=mv, in_=stats)
            ve = p2.tile([NQ, 1], F32, tag="ve2")
            nc.vector.tensor_scalar_add(out=ve, in0=mv[:, 1:2], scalar1=EPS)
            rstd = rsqrt(ve, NQ, "p2")
            ctxln = p2.tile([NQ, H], BF16, tag="ctxln")
            if "gcln" in skip:
                nc.vector.tensor_scalar(out=ctxln, in0=cps, scalar1=mv[:, 0:1],
                                        scalar2=rstd,
                                        op0=mybir.AluOpType.subtract,
                                        op1=mybir.AluOpType.mult)
            else:
                cn = p2.tile([NQ, H], F32, tag="cn")
                nc.vector.tensor_scalar(out=cn, in0=cps, scalar1=mv[:, 0:1],
                                        scalar2=rstd,
                                        op0=mybir.AluOpType.subtract,
                                        op1=mybir.AluOpType.mult)
                cgn = p2.tile([NQ, H], F32, tag="cgn")
                nc.vector.tensor_mul(out=cgn, in0=cn, in1=cg_sb)
                nc.vector.tensor_add(out=ctxln, in0=cgn, in1=cb_sb)

            if warm:
                keep_warm(warm)
            pt = pst.tile([P, 4, NQ + 1], BF16, tag="tr")
            for j in range(4):
                nc.tensor.transpose(pt[:, j, 0:NQ], ctxln[:, j * P:(j + 1) * P],
                                    id17)
            gcol = p2.tile([P, 4, 1], F32, tag="gcol")
            nc.vector.tensor_copy(out=gcol, in_=pt[:, :, 0:1])
            fusedT = p2.tile([P, 8, D], BF16, tag="fusedT")
            nc.vector.tensor_copy(out=fusedT[:, 0:4, :], in_=pt[:, :, 1:1 + D])
            for c in range(4):
                nc.vector.tensor_scalar_mul(out=fusedT[:, 4 + c, :], in0=ones_col,
                                            scalar1=gcol[:, c, :])
            return fusedT

        def row_ffn(b, fusedT, warm=0):
            ph1 = ffnp.tile([D, H], F32, tag="ph")
            for kc in range(8):
                nc.tensor.matmul(ph1, lhsT=fusedT[:, kc, :],
                                 rhs=w1_sb[:, kc, :], start=(kc == 0),
                                 stop=(kc == 7 and "bf1" in skip))
            if "bf1" not in skip:
                nc.tensor.matmul(ph1, lhsT=ones_row[:, :D], rhs=bf1_sb,
                                 start=False, stop=True)
            h1 = p2.tile([D, H], F32, tag="h1")
            nc.scalar.activation(out=h1, in_=ph1,
                                 func=mybir.ActivationFunctionType.Gelu)

            stats = p2.tile([D, 6], F32, tag="stats3")
            nc.vector.bn_stats(out=stats, in_=h1)
            mv = p2.tile([D, 2], F32, tag="mv3")
            nc.vector.bn_aggr(out=mv, in_=stats)
            ve = p2.tile([D, 1], F32, tag="ve3")
            nc.vector.tensor_scalar_add(out=ve, in0=mv[:, 1:2], scalar1=EPS)
            rstd = rsqrt(ve, D, "p3")
            h1ln = p2.tile([D, H], BF16, tag="h1ln")
            if "fln" in skip:
                nc.vector.tensor_scalar(out=h1ln, in0=h1, scalar1=mv[:, 0:1],
                                        scalar2=rstd,
                                        op0=mybir.AluOpType.subtract,
                                        op1=mybir.AluOpType.mult)
            else:
                hn = p2.tile([D, H], F32, tag="hn")
                nc.vector.tensor_scalar(out=hn, in0=h1, scalar1=mv[:, 0:1],
                                        scalar2=rstd,
                                        op0=mybir.AluOpType.subtract,
                                        op1=mybir.AluOpType.mult)
                hg = p2.tile([D, H], F32, tag="hg")
                nc.vector.tensor_mul(out=hg, in0=hn, in1=fg_sb)
                nc.vector.tensor_add(out=h1ln, in0=hg, in1=fb_sb)

            if warm:
                keep_warm(warm)
            pt = pst.tile([P, 4, NQ + 1], BF16, tag="tr")
            for j in range(4):
                nc.tensor.transpose(pt[:, j, 0:D], h1ln[:, j * P:(j + 1) * P],
                                    id16)
            h1T = p2.tile([P, 4, D], BF16, tag="h1T")
            nc.vector.tensor_copy(out=h1T, in_=pt[:, :, 0:D])

            ph2 = ffnp.tile([D, H], F32, tag="ph")
            for kc in range(4):
                nc.tensor.matmul(ph2, lhsT=h1T[:, kc, :], rhs=w2_sb[:, kc, :],
                                 start=(kc == 0),
                                 stop=(kc == 3 and "bf2" in skip))
            if "bf2" not in skip:
                nc.tensor.matmul(ph2, lhsT=ones_row[:, :D], rhs=bf2_sb,
                                 start=False, stop=True)
            h2 = p2.tile([D, H], F32, tag="h2")
            nc.scalar.activation(out=h2, in_=ph2,
                                 func=mybir.ActivationFunctionType.Gelu)

            prod = p2.tile([D, H], F32, tag="prod")
            nc.vector.tensor_mul(out=prod, in0=h2, in1=wo_sb)
            scr = p2.tile([D, H], F32, tag="scr")
            lsum = p2.tile([D, 1], F32, tag="lsum")
            nc.scalar.activation(out=scr, in_=prod,
                                 func=mybir.ActivationFunctionType.Copy,
                                 accum_out=lsum)
            nc.vector.tensor_scalar_add(out=logit_sb[:, b:b + 1], in0=lsum,
                                        scalar1=float(b_out_s))

        # ---- driver: pipelined projection stream with interleaved phase 2.
        # ctx matmuls trail the projection by two tiles (slack for the exp
        # chain and for ACT table switches); each completed row's LN +
        # transposes + FFN run inline so only the last row drains at the end.
        def keep_warm(n):
            wtile = psx.tile([P, 2 * H], F32, tag="px", name="warm")
            for i in range(n):
                nc.tensor.matmul(wtile[:, 0:H], lhsT=x_sb[:, 0, 0:P],
                                 rhs=x_sb[:, 1, :], start=True, stop=True,
                                 skip_group_check=True)

        done_rows = set()
        for t in range(TT):
            proj(t)
            if t + 6 < TT:
                dma_tile(t + 6)
            post(t)
            if t >= 2:
                ctx_mm(t - 2)
            if t >= 3 and (t - 3) % NT == NT - 1:
                b = (t - 3) // NT
                row_ffn(b, row_ctx(b))
                done_rows.add(b)
        ctx_mm(TT - 2)
        keep_warm(10)
        ctx_mm(TT - 1)
        for b in range(B_LOCAL):
            if b not in done_rows:
                fused = row_ctx(b, warm=12)
                row_ffn(b, fused, warm=14)

        nc.sync.dma_start(out=out.ap(), in_=logit_sb)


def _np(x):
    return np.asarray(x)


LAST_RESULT = None


def kernel(**inputs):
    from concourse.bass_utils import run_bass_kernel_spmd

    token_states = _np(inputs["token_states"]).astype(np.float32)
    mask = _np(inputs["attention_mask"])
    W_proj = _np(inputs["W_proj"]).astype(np.float32)
    b_proj = _np(inputs["b_proj"]).astype(np.float32)
    tln_g = _np(inputs["tln_g"]).astype(np.float32)
    tln_b = _np(inputs["tln_b"]).astype(np.float32)
    gln_g = _np(inputs["gln_g"]).astype(np.float32)
    gln_b = _np(inputs["gln_b"]).astype(np.float32)
    cln_g = _np(inputs["cln_g"]).astype(np.float32)
    cln_b = _np(inputs["cln_b"]).astype(np.float32)
    fln_g = _np(inputs["fln_g"]).astype(np.float32)
    fln_b = _np(inputs["fln_b"]).astype(np.float32)
    domain_queries = _np(inputs["domain_queries"]).astype(np.float32)
    global_query = _np(inputs["global_query"]).astype(np.float32)
    W_ff1 = _np(inputs["W_ff1"]).astype(np.float32)
    b_ff1 = _np(inputs["b_ff1"]).astype(np.float32)
    W_ff2 = _np(inputs["W_ff2"]).astype(np.float32)
    b_ff2 = _np(inputs["b_ff2"]).astype(np.float32)
    W_out = _np(inputs["W_out"]).astype(np.float32)
    b_out = _np(inputs["b_out"]).astype(np.float32)
    log_temperature = _np(inputs["log_temperature"]).astype(np.float32)

    Bq, Sq = mask.shape
    assert (Bq, Sq) == (B, S) and token_states.shape == (B, S, TOKD)

    # ---- host preprocessing ----
    compact = os.environ.get("KERNEL_COMPACT", "1") == "1"
    if compact:
        counts = mask.astype(bool).sum(axis=1)
        S_c = int(max(P, -(-int(counts.max()) // P) * P))
    else:
        S_c = S

    ts_c = np.zeros((B, S_c, TOKD), np.float32)
    padbias = np.full((B, S_c), -1e9, np.float32)
    if compact:
        for b in range(B):
            idx = np.flatnonzero(mask[b])
            n = len(idx)
            ts_c[b, :n] = token_states[b, idx]
            padbias[b, :n] = 0.0
    else:
        ts_c[:] = token_states
        padbias[:] = np.where(mask != 0, 0.0, -1e9)

    temp = float(np.clip(np.exp(log_temperature[0]), 0.3, 3.0))
    inv_t = 1.0 / temp
    wo_host = (W_out[:, 0] * inv_t).astype(np.float32)
    b_out_s = float(b_out[0] * inv_t)

    # queries folded into the projection: row 0 = global, 1..16 = domains
    q_all = np.concatenate([global_query[None, :], domain_queries], 0)  # [17,H]
    q_eff = q_all * tln_g[None, :]                                      # [17,H]
    sq = q_eff.sum(axis=1)                                              # [17]
    w_bar = W_proj.mean(axis=1)                                         # [TOKD]
    Q_hat = W_proj @ q_eff.T - w_bar[:, None] * sq[None, :]             # [TOKD,17]
    # pre-rstd per-query score offset from the projection bias. (The LN-bias
    # term tln_b.q is a post-rstd per-query constant — a uniform softmax
    # rescale per query — and cancels in the context LN, so it's dropped.)
    bq = (b_proj @ q_eff.T) - float(b_proj.mean()) * sq

    W_aug = np.concatenate([W_proj, Q_hat], axis=1)                 # [TOKD,529]
    w_host = W_aug.reshape(KC, P, WTOT).transpose(1, 0, 2)          # [128,KC,529]
    bprow_host = np.concatenate([b_proj, bq]).reshape(1, WTOT)

    cg_host = np.ones((NQ, H), np.float32)
    cb_host = np.zeros((NQ, H), np.float32)
    cg_host[0] = gln_g
    cb_host[0] = gln_b
    cg_host[1:] = cln_g
    cb_host[1:] = cln_b

    bf16 = ml_dtypes.bfloat16

    skip = set()
    if np.all(tln_g == 1) and np.all(tln_b == 0):
        skip.add("tln")
    if np.all(cg_host == 1) and np.all(cb_host == 0):
        skip.add("gcln")
    if np.all(fln_g == 1) and np.all(fln_b == 0):
        skip.add("fln")
    if np.all(b_proj == 0) and np.all(bq == 0):
        skip.add("bp")
    if np.all(b_ff1 == 0):
        skip.add("bf1")
    if np.all(b_ff2 == 0):
        skip.add("bf2")

    nc = build_nc(S_c, b_out_s, frozenset(skip))

    NT = S_c // P
    TT = B_LOCAL * NT

    shared = dict(
        wp=w_host.reshape(P, KC * WTOT).astype(bf16),
        bprow=bprow_host.astype(bf16),
        tg=tln_g[None, :], tb=tln_b[None, :],
        cg=cg_host, cb=cb_host,
        fg=fln_g[None, :], fb=fln_b[None, :],
        w1=W_ff1.reshape(8, P, H).transpose(1, 0, 2).reshape(P, 8 * H).astype(bf16),
        bf1=b_ff1[None, :].astype(bf16),
        w2=W_ff2.reshape(4, P, H).transpose(1, 0, 2).reshape(P, 4 * H).astype(bf16),
        bf2=b_ff2[None, :].astype(bf16),
        wo=wo_host[None, :],
    )

    in_maps = []
    for c in range(N_CORES):
        m = dict(shared)
        bs = slice(c * B_LOCAL, (c + 1) * B_LOCAL)
        tsc = ts_c[bs].reshape(TT, P, KC, P)          # [tile, s, c, p]
        m["ts"] = np.ascontiguousarray(
            tsc.transpose(0, 3, 2, 1)).reshape(TT * P, TOKD).astype(bf16)
        m["pb"] = np.ascontiguousarray(
            padbias[bs].reshape(TT, P).T).astype(np.float32)
        in_maps.append(m)

    trace = os.environ.get("KERNEL_TRACE", "0") == "1"
    kw = {}
    if trace:
        kw = dict(trace=True, tmpdir=os.environ.get("KERNEL_TRACE_DIR") or None)
    res = run_bass_kernel_spmd(nc, in_maps, core_ids=list(range(N_CORES)), **kw)
    global LAST_RESULT
    LAST_RESULT = res
    outs = [res.results[c]["out"].T for c in range(N_CORES)]
    return np.concatenate(outs, axis=0).astype(np.float32)


if __name__ == "__main__":
    pass


# revision 23
# speedup vs baseline: 1.0191x; 1.0191x over previous
"""Trainium2 Bass kernel for nn_MetaRouter (dense_transformer).

Contract: kernel(**inputs) takes FULL unsharded inputs (as produced by
reference.setup_inputs()) and returns the FULL [B, D] logits, matching
reference.reference(**inputs).

Strategy:
  - Data-parallel over batch: B=16 split as 2 rows per core x 8 cores.
    All parameters replicated. No collectives.
  - Host side: tokens with attention_mask==0 get softmax weight exactly 0
    for every query, so each row is compacted to its unmasked tokens
    (padded to a multiple of 128; pad slots get a -1e9 score bias).
    ts is pre-cast to bf16 and pre-transposed into [tile, 128 feat, tok]
    chunks so the chip never transposes it. The 17 attention queries are
    folded into the projection weight matrix as extra columns:
        Q_hat = W @ q - w_bar * colsum(q)   (w_bar = row-mean of W)
    which makes raw_score[s,q] = ts_s . Q_hat[:,q] = v_s.q - mu_s*sum(q),
    i.e. the LN mean-correction is pre-applied; only the per-token rstd
    scaling remains. So scores cost 17 extra matmul columns, not a
    separate pass, and x^T never needs to exist on chip.
  - Softmax denominators and the LN mean shift both cancel inside the
    downstream LayerNorms (LN is invariant to positive scaling and
    uniform shifts), so the context sums use unnormalized exp weights
    against the raw (pre-LN) projections, with the per-token rstd folded
    into the exp weights. No reduce_max, no reciprocal, no renorm.
  - Per tile of 128 tokens: 64 matmuls (32 k-chunks x 2 PSUM splits of
    265+264 columns; one PSUM bank each, LDWEIGHTS fully hidden), then
    DVE does bn_stats/bn_aggr + a bitcast-Newton rsqrt (keeps the ACT
    table pinned on Exp), ACT does exp(rstd*raw + padbias) in a single
    fused instruction, and the per-row context accumulates incrementally
    in PSUM via one [128,17]x[128,512] matmul interleaved into the
    projection stream.
  - Tail per row: LN (DVE rsqrt), PE transposes for the FFN operands,
    gelu-FFN with all weights pre-chunked, output head folded with the
    temperature on the host.
"""

import os

import numpy as np
import ml_dtypes

import concourse.bass as bass
import concourse.bacc as bacc
import concourse.tile as tile
from concourse import mybir
from concourse.masks import make_identity

P = 128
H = 512
TOKD = 4096
KC = TOKD // P    # 32 k-chunks of the projection contraction
NQ = 17           # 1 global + 16 domain queries
WTOT = H + NQ     # 529 projection output columns
SPL = 256         # W column split; region B holds W[256:] + scores
D = 16
B = 16
S = 2048
N_CORES = 8
B_LOCAL = B // N_CORES
EPS = 1e-5
F32 = mybir.dt.float32
I32 = mybir.dt.int32
BF16 = mybir.dt.bfloat16
MAGIC = 0x5F3759DF


def build_nc(S_c: int, b_out_s: float, skip=frozenset()):
    """Build the per-core Bass program for padded/compacted seq length S_c."""
    assert S_c % P == 0
    NT = S_c // P          # token tiles per batch row
    TT = B_LOCAL * NT      # token tiles per core

    nc = bacc.Bacc("TRN2", target_bir_lowering=False, num_swdge_queues=2)

    ts = nc.declare_dram_parameter("ts", [TT * P, TOKD], BF16, isOutput=False)
    pb = nc.declare_dram_parameter("pb", [P, TT], F32, isOutput=False)
    wp = nc.declare_dram_parameter("wp", [P, KC * WTOT], BF16, isOutput=False)
    bprow = nc.declare_dram_parameter("bprow", [1, WTOT], BF16, isOutput=False)
    tg = nc.declare_dram_parameter("tg", [1, H], F32, isOutput=False)
    tb = nc.declare_dram_parameter("tb", [1, H], F32, isOutput=False)
    cg = nc.declare_dram_parameter("cg", [NQ, H], F32, isOutput=False)
    cb = nc.declare_dram_parameter("cb", [NQ, H], F32, isOutput=False)
    fg = nc.declare_dram_parameter("fg", [1, H], F32, isOutput=False)
    fb = nc.declare_dram_parameter("fb", [1, H], F32, isOutput=False)
    w1 = nc.declare_dram_parameter("w1", [P, 8 * H], BF16, isOutput=False)
    bf1 = nc.declare_dram_parameter("bf1", [1, H], BF16, isOutput=False)
    w2 = nc.declare_dram_parameter("w2", [P, 4 * H], BF16, isOutput=False)
    bf2 = nc.declare_dram_parameter("bf2", [1, H], BF16, isOutput=False)
    wo = nc.declare_dram_parameter("wo", [1, H], F32, isOutput=False)
    out = nc.declare_dram_parameter("out", [D, B_LOCAL], F32, isOutput=True)

    with tile.TileContext(nc) as tc:
        _emit(tc, nc, NT, TT, b_out_s, skip,
              ts=ts, pb=pb, wp=wp, bprow=bprow, tg=tg, tb=tb, cg=cg, cb=cb,
              fg=fg, fb=fb, w1=w1, bf1=bf1, w2=w2, bf2=bf2, wo=wo, out=out)
    nc.compile()
    return nc


def _emit(tc, nc, NT, TT, b_out_s, skip, *, ts, pb, wp, bprow, tg, tb, cg, cb,
          fg, fb, w1, bf1, w2, bf2, wo, out):
    from contextlib import ExitStack
    ctx = ExitStack()
    with ctx:
        const = ctx.enter_context(tc.tile_pool(name="const", bufs=1))
        tsp = ctx.enter_context(tc.tile_pool(name="tsp", bufs=6))
        xp = ctx.enter_context(tc.tile_pool(name="xp", bufs=1))
        lnp = ctx.enter_context(tc.tile_pool(name="lnp", bufs=2))
        p2 = ctx.enter_context(tc.tile_pool(name="p2", bufs=2))
        psx = ctx.enter_context(tc.tile_pool(name="psx", bufs=2, space="PSUM"))
        ctxp = ctx.enter_context(tc.tile_pool(name="ctxp", bufs=1, space="PSUM"))
        pst = ctx.enter_context(tc.tile_pool(name="pst", bufs=1, space="PSUM"))
        ffnp = ctx.enter_context(tc.tile_pool(name="ffnp", bufs=2, space="PSUM"))

        # ---- weights / first ts tiles lead the DMA rings ----
        # W' goes chunk-by-chunk round-robin over three HWDGE rings so the
        # k-loop of tile 0 never waits on a monolithic transfer; padbias
        # leads the vector ring (needed by post(0)).
        w_sb = const.tile([P, KC, WTOT], BF16)
        _wp = wp.ap().rearrange("p (c w) -> p c w", w=WTOT)
        pb_sb = const.tile([P, TT], F32)
        nc.sync.dma_start(out=pb_sb, in_=pb.ap())
        # Startup is DMA-bound on W' (4.3MB) + the first two ts tiles: spread
        # W' as 2-chunk blocks round-robin over all three DMA paths, with the
        # first tiles' halves slotted after each path's first block. Tiles 0/1
        # are then consumed k-block-major so the PE follows the arrival order.
        prefetched = {}

        def dma_tile(t, eng=None):
            if eng is None:
                eng = (nc.gpsimd, nc.sync, nc.scalar)[t % 3]
            tt = tsp.tile([P, KC * P], BF16, tag="ts")
            eng.dma_start(out=tt, in_=ts.ap()[t * P:(t + 1) * P, :])
            prefetched[t] = tt

        # Ring DMAs hit ~280GB/s only with large contiguous per-partition
        # lines, so W' goes in four 8-chunk blocks (8.5KB lines) split across
        # both rings while ts0 leads the other ring and SWDGE carries ts1/ts2.
        dma_tile(0, nc.scalar)
        nc.sync.dma_start(out=w_sb[:, 0:8, :], in_=_wp[:, 0:8, :])
        dma_tile(1, nc.gpsimd)
        dma_tile(2, nc.gpsimd)
        nc.scalar.dma_start(out=w_sb[:, 8:16, :], in_=_wp[:, 8:16, :])
        nc.sync.dma_start(out=w_sb[:, 16:24, :], in_=_wp[:, 16:24, :])
        nc.scalar.dma_start(out=w_sb[:, 24:32, :], in_=_wp[:, 24:32, :])
        dma_tile(3, nc.sync)
        dma_tile(4, nc.scalar)
        dma_tile(5, nc.gpsimd)
        # ---- constants ----
        def bcast(dram, parts, dt=F32):
            t = const.tile([parts, H], dt, tag=f"c_{dram.name}")
            a = dram.ap()
            nc.sync.dma_start(
                out=t, in_=bass.AP(tensor=a.tensor, offset=a.offset,
                                   ap=[[0, parts]] + list(a.ap[1:])))
            return t

        w1_sb = const.tile([P, 8, H], BF16)
        nc.sync.dma_start(out=w1_sb, in_=w1.ap().rearrange("p (c h) -> p c h", h=H))
        w2_sb = const.tile([P, 4, H], BF16)
        nc.sync.dma_start(out=w2_sb, in_=w2.ap().rearrange("p (c h) -> p c h", h=H))
        wo_sb = bcast(wo, D)
        if "tln" not in skip:
            tg_sb = bcast(tg, P)
            tb_sb = bcast(tb, P)
        if "gcln" not in skip:
            cg_sb = const.tile([NQ, H], F32)
            nc.sync.dma_start(out=cg_sb, in_=cg.ap())
            cb_sb = const.tile([NQ, H], F32)
            nc.sync.dma_start(out=cb_sb, in_=cb.ap())
        if "fln" not in skip:
            fg_sb = bcast(fg, D)
            fb_sb = bcast(fb, D)
        if "bf1" not in skip:
            bf1_sb = const.tile([1, H], BF16)
            nc.sync.dma_start(out=bf1_sb, in_=bf1.ap())
        if "bf2" not in skip:
            bf2_sb = const.tile([1, H], BF16)
            nc.sync.dma_start(out=bf2_sb, in_=bf2.ap())
        if "bp" not in skip:
            bprow_sb = const.tile([1, WTOT], BF16)
            nc.sync.dma_start(out=bprow_sb, in_=bprow.ap())

        ones_row = const.tile([1, P], BF16)
        nc.vector.memset(ones_row, 1.0)
        ones_col = const.tile([P, D], BF16)
        nc.vector.memset(ones_col, 1.0)
        id17 = const.tile([NQ, NQ], BF16)
        make_identity(nc, id17)
        id16 = const.tile([D, D], BF16)
        make_identity(nc, id16)
        magic = const.tile([P, 1], I32)
        nc.vector.memset(magic, MAGIC)

        # x (raw projection, bf16) + unnormalized-attn weights, SBUF-resident
        x_sb = xp.tile([P, TT, H], BF16)
        pexpT = xp.tile([P, TT, NQ], BF16)
        logit_sb = xp.tile([D, B_LOCAL], F32)

        def rsqrt(ve, parts, tag, iters=1):
            """y ~= (ve)^-0.5 via bitcast seed + Newton steps (DVE only)."""
            y = lnp.tile([parts, 1], F32, tag=f"y_{tag}")
            sh = lnp.tile([parts, 1], I32, tag=f"sh_{tag}")
            nc.vector.tensor_scalar(out=sh, in0=ve.bitcast(I32), scalar1=1,
                                    scalar2=None,
                                    op0=mybir.AluOpType.arith_shift_right)
            nc.vector.tensor_tensor(out=y.bitcast(I32), in0=magic[:parts],
                                    in1=sh, op=mybir.AluOpType.subtract)
            t1 = lnp.tile([parts, 1], F32, tag=f"t1_{tag}")
            hh = lnp.tile([parts, 1], F32, tag=f"h_{tag}")
            for _ in range(iters):
                nc.vector.tensor_mul(out=t1, in0=y, in1=y)
                nc.vector.tensor_mul(out=t1, in0=t1, in1=ve)
                nc.vector.tensor_scalar(out=hh, in0=t1, scalar1=-0.5,
                                        scalar2=1.5, op0=mybir.AluOpType.mult,
                                        op1=mybir.AluOpType.add)
                nc.vector.tensor_mul(out=y, in0=y, in1=hh)
            return y

        psums = {}

        def proj_alloc(t):
            tsT = prefetched.pop(t).rearrange("p (c s) -> p c s", s=P)
            px = psx.tile([P, 2 * H], F32, tag="px", name="px")
            psums[t] = px
            return tsT, px

        def proj_chunk(tsT, px, k, t):
            first, last = k == 0, k == KC - 1 and "bp" in skip
            nc.tensor.matmul(px[:, 0:SPL], lhsT=tsT[:, k, :],
                             rhs=w_sb[:, k, 0:SPL], start=first, stop=last,
                             skip_group_check=True)
            nc.tensor.matmul(px[:, H:H + WTOT - SPL], lhsT=tsT[:, k, :],
                             rhs=w_sb[:, k, SPL:], start=first, stop=last,
                             skip_group_check=True)

        def proj_bias(px):
            if "bp" not in skip:
                nc.tensor.matmul(px[:, 0:SPL], lhsT=ones_row,
                                 rhs=bprow_sb[:, 0:SPL], start=False,
                                 stop=True, skip_group_check=True)
                nc.tensor.matmul(px[:, H:H + WTOT - SPL], lhsT=ones_row,
                                 rhs=bprow_sb[:, SPL:], start=False,
                                 stop=True, skip_group_check=True)

        def proj(t):
            tsT, px = proj_alloc(t)
            for k in range(KC):
                proj_chunk(tsT, px, k, t)
            proj_bias(px)

        def post(t):
            """Stats + x store + exp-weights for tile t (DVE/ACT work)."""
            px = psums.pop(t)
            vreg = px.rearrange("p (r x) -> p r x", x=H)[:, :, 0:SPL]
            stats = lnp.tile([P, 12], F32, tag="stats")
            nc.vector.bn_stats(out=stats[:, 0:6], in_=vreg[:, 0, :])
            nc.vector.bn_stats(out=stats[:, 6:12], in_=vreg[:, 1, :])
            mv = lnp.tile([P, 2], F32, tag="mv")
            nc.vector.bn_aggr(out=mv, in_=stats)
            ve = lnp.tile([P, 1], F32, tag="ve")
            nc.vector.tensor_scalar_add(out=ve, in0=mv[:, 1:2], scalar1=EPS)
            rstd = rsqrt(ve, P, "p1")
            # attn weights first: they gate the trailing ctx matmul
            nc.scalar.activation(out=pexpT[:, t, :], in_=px[:, H + H - SPL:H + WTOT - SPL],
                                 func=mybir.ActivationFunctionType.Exp,
                                 bias=pb_sb[:, t:t + 1], scale=rstd)
            xv = x_sb[:, t, :].rearrange("p (r x) -> p r x", x=SPL)
            if "tln" in skip:
                nc.vector.tensor_scalar_mul(out=pexpT[:, t, :],
                                            in0=pexpT[:, t, :], scalar1=rstd)
                # store raw v; rstd folds into the attn weights, mu cancels
                nc.vector.tensor_copy(out=xv, in_=vreg)
            else:
                xa = lnp.tile([P, H], F32, tag="xa")
                nc.vector.tensor_scalar(
                    out=xa.rearrange("p (r x) -> p r x", x=SPL), in0=vreg,
                    scalar1=mv[:, 0:1], scalar2=rstd,
                    op0=mybir.AluOpType.subtract, op1=mybir.AluOpType.mult)
                xg = lnp.tile([P, H], F32, tag="xg")
                nc.vector.tensor_mul(out=xg, in0=xa, in1=tg_sb)
                nc.vector.tensor_add(out=x_sb[:, t, :], in0=xg, in1=tb_sb)

        ctx_ps = {}

        def ctx_mm(t):
            b, i = divmod(t, NT)
            if i == 0:
                ctx_ps[b] = ctxp.tile([NQ, H], F32, tag="ctx", name="ctx")
            nc.tensor.matmul(ctx_ps[b], lhsT=pexpT[:, t, :], rhs=x_sb[:, t, :],
                             start=(i == 0), stop=(i == NT - 1))

        def row_ctx(b, warm=0):
            """Context LN + transposes + fused operand build for row b."""
            cps = ctx_ps.pop(b)
            stats = p2.tile([NQ, 6], F32, tag="stats2")
            nc.vector.bn_stats(out=stats, in_=cps)
            mv = p2.tile([NQ, 2], F32, tag="mv2")
            nc.vector.bn_aggr(out=mv, in_=stats)
            ve = p2.tile([NQ, 1], F32, tag="ve2")
            nc.vector.tensor_scalar_add(out=ve, in0=mv[:, 1:2], scalar1=EPS)
            rstd = rsqrt(ve, NQ, "p2")
            ctxln = p2.tile([NQ, H], BF16, tag="ctxln")
            if "gcln" in skip:
                nc.vector.tensor_scalar(out=ctxln, in0=cps, scalar1=mv[:, 0:1],
                                        scalar2=rstd,
                                        op0=mybir.AluOpType.subtract,
                                        op1=mybir.AluOpType.mult)
            else:
                cn = p2.tile([NQ, H], F32, tag="cn")
                nc.vector.tensor_scalar(out=cn, in0=cps, scalar1=mv[:, 0:1],
                                        scalar2=rstd,
                                        op0=mybir.AluOpType.subtract,
                                        op1=mybir.AluOpType.mult)
                cgn = p2.tile([NQ, H], F32, tag="cgn")
                nc.vector.tensor_mul(out=cgn, in0=cn, in1=cg_sb)
                nc.vector.tensor_add(out=ctxln, in0=cgn, in1=cb_sb)

            if warm:
                keep_warm(warm)
            pt = pst.tile([P, 4, NQ + 1], BF16, tag="tr")
            for j in range(4):
                nc.tensor.transpose(pt[:, j, 0:NQ], ctxln[:, j * P:(j + 1) * P],
                                    id17)
            gcol = p2.tile([P, 4, 1], F32, tag="gcol")
            nc.vector.tensor_copy(out=gcol, in_=pt[:, :, 0:1])
            fusedT = p2.tile([P, 8, D], BF16, tag="fusedT")
            nc.vector.tensor_copy(out=fusedT[:, 0:4, :], in_=pt[:, :, 1:1 + D])
            for c in range(4):
                nc.vector.tensor_scalar_mul(out=fusedT[:, 4 + c, :], in0=ones_col,
                                            scalar1=gcol[:, c, :])
            return fusedT

        def row_ffn(b, fusedT, warm=0):
            ph1 = ffnp.tile([D, H], F32, tag="ph")
            for kc in range(8):
                nc.tensor.matmul(ph1, lhsT=fusedT[:, kc, :],
                                 rhs=w1_sb[:, kc, :], start=(kc == 0),
                                 stop=(kc == 7 and "bf1" in skip))
            if "bf1" not in skip:
                nc.tensor.matmul(ph1, lhsT=ones_row[:, :D], rhs=bf1_sb,
                                 start=False, stop=True)
            h1 = p2.tile([D, H], F32, tag="h1")
            nc.scalar.activation(out=h1, in_=ph1,
                                 func=mybir.ActivationFunctionType.Gelu)

            stats = p2.tile([D, 6], F32, tag="stats3")
            nc.vector.bn_stats(out=stats, in_=h1)
            mv = p2.tile([D, 2], F32, tag="mv3")
            nc.vector.bn_aggr(out=mv, in_=stats)
            ve = p2.tile([D, 1], F32, tag="ve3")
            nc.vector.tensor_scalar_add(out=ve, in0=mv[:, 1:2], scalar1=EPS)
            rstd = rsqrt(ve, D, "p3")
            h1ln = p2.tile([D, H], BF16, tag="h1ln")
            if "fln" in skip:
                nc.vector.tensor_scalar(out=h1ln, in0=h1, scalar1=mv[:, 0:1],
                                        scalar2=rstd,
                                        op0=mybir.AluOpType.subtract,
                                        op1=mybir.AluOpType.mult)
            else:
                hn = p2.tile([D, H], F32, tag="hn")
                nc.vector.tensor_scalar(out=hn, in0=h1, scalar1=mv[:, 0:1],
                                        scalar2=rstd,
                                        op0=mybir.AluOpType.subtract,
                                        op1=mybir.AluOpType.mult)
                hg = p2.tile([D, H], F32, tag="hg")
                nc.vector.tensor_mul(out=hg, in0=hn, in1=fg_sb)
                nc.vector.tensor_add(out=h1ln, in0=hg, in1=fb_sb)

            if warm:
                keep_warm(warm)
            pt = pst.tile([P, 4, NQ + 1], BF16, tag="tr")
            for j in range(4):
                nc.tensor.transpose(pt[:, j, 0:D], h1ln[:, j * P:(j + 1) * P],
                                    id16)
            h1T = p2.tile([P, 4, D], BF16, tag="h1T")
            nc.vector.tensor_copy(out=h1T, in_=pt[:, :, 0:D])

            ph2 = ffnp.tile([D, H], F32, tag="ph")
            for kc in range(4):
                nc.tensor.matmul(ph2, lhsT=h1T[:, kc, :], rhs=w2_sb[:, kc, :],
                                 start=(kc == 0),
                                 stop=(kc == 3 and "bf2" in skip))
            if "bf2" not in skip:
                nc.tensor.matmul(ph2, lhsT=ones_row[:, :D], rhs=bf2_sb,
                                 start=False, stop=True)
            h2 = p2.tile([D, H], F32, tag="h2")
            nc.scalar.activation(out=h2, in_=ph2,
                                 func=mybir.ActivationFunctionType.Gelu)

            prod = p2.tile([D, H], F32, tag="prod")
            nc.vector.tensor_mul(out=prod, in0=h2, in1=wo_sb)
            scr = p2.tile([D, H], F32, tag="scr")
            lsum = p2.tile([D, 1], F32, tag="lsum")
            nc.scalar.activation(out=scr, in_=prod,
                                 func=mybir.ActivationFunctionType.Copy,
                                 accum_out=lsum)
            nc.vector.tensor_scalar_add(out=logit_sb[:, b:b + 1], in0=lsum,
                                        scalar1=float(b_out_s))

        # ---- driver: pipelined projection stream with interleaved phase 2.
        # ctx matmuls trail the projection by two tiles (slack for the exp
        # chain and for ACT table switches); each completed row's LN +
        # transposes + FFN run inline so only the last row drains at the end.
        def keep_warm(n):
            wtile = psx.tile([P, 2 * H], F32, tag="px", name="warm")
            for i in range(n):
                nc.tensor.matmul(wtile[:, 0:H], lhsT=x_sb[:, 0, 0:P],
                                 rhs=x_sb[:, 1, :], start=True, stop=True,
                                 skip_group_check=True)

        done_rows = set()
        for t in range(TT):
            proj(t)
            if t + 6 < TT:
                dma_tile(t + 6)
            post(t)
            if t >= 2:
                ctx_mm(t - 2)
            if t >= 3 and (t - 3) % NT == NT - 1:
                b = (t - 3) // NT
                row_ffn(b, row_ctx(b))
                done_rows.add(b)
        ctx_mm(TT - 2)
        keep_warm(10)
        ctx_mm(TT - 1)
        for b in range(B_LOCAL):
            if b not in done_rows:
                fused = row_ctx(b, warm=12)
                row_ffn(b, fused, warm=14)

        nc.sync.dma_start(out=out.ap(), in_=logit_sb)


def _np(x):
    return np.asarray(x)


LAST_RESULT = None


def kernel(**inputs):
    from concourse.bass_utils import run_bass_kernel_spmd

    token_states = _np(inputs["token_states"]).astype(np.float32)
    mask = _np(inputs["attention_mask"])
    W_proj = _np(inputs["W_proj"]).astype(np.float32)
    b_proj = _np(inputs["b_proj"]).astype(np.float32)
    tln_g = _np(inputs["tln_g"]).astype(np.float32)
    tln_b = _np(inputs["tln_b"]).astype(np.float32)
    gln_g = _np(inputs["gln_g"]).astype(np.float32)
    gln_b = _np(inputs["gln_b"]).astype(np.float32)
    cln_g = _np(inputs["cln_g"]).astype(np.float32)
    cln_b = _np(inputs["cln_b"]).astype(np.float32)
    fln_g = _np(inputs["fln_g"]).astype(np.float32)
    fln_b = _np(inputs["fln_b"]).astype(np.float32)
    domain_queries = _np(inputs["domain_queries"]).astype(np.float32)
    global_query = _np(inputs["global_query"]).astype(np.float32)
    W_ff1 = _np(inputs["W_ff1"]).astype(np.float32)
    b_ff1 = _np(inputs["b_ff1"]).astype(np.float32)
    W_ff2 = _np(inputs["W_ff2"]).astype(np.float32)
    b_ff2 = _np(inputs["b_ff2"]).astype(np.float32)
    W_out = _np(inputs["W_out"]).astype(np.float32)
    b_out = _np(inputs["b_out"]).astype(np.float32)
    log_temperature = _np(inputs["log_temperature"]).astype(np.float32)

    Bq, Sq = mask.shape
    assert (Bq, Sq) == (B, S) and token_states.shape == (B, S, TOKD)

    # ---- host preprocessing ----
    compact = os.environ.get("KERNEL_COMPACT", "1") == "1"
    if compact:
        counts = mask.astype(bool).sum(axis=1)
        S_c = int(max(P, -(-int(counts.max()) // P) * P))
    else:
        S_c = S

    ts_c = np.zeros((B, S_c, TOKD), np.float32)
    padbias = np.full((B, S_c), -1e9, np.float32)
    if compact:
        for b in range(B):
            idx = np.flatnonzero(mask[b])
            n = len(idx)
            ts_c[b, :n] = token_states[b, idx]
            padbias[b, :n] = 0.0
    else:
        ts_c[:] = token_states
        padbias[:] = np.where(mask != 0, 0.0, -1e9)

    temp = float(np.clip(np.exp(log_temperature[0]), 0.3, 3.0))
    inv_t = 1.0 / temp
    wo_host = (W_out[:, 0] * inv_t).astype(np.float32)
    b_out_s = float(b_out[0] * inv_t)

    # queries folded into the projection: row 0 = global, 1..16 = domains
    q_all = np.concatenate([global_query[None, :], domain_queries], 0)  # [17,H]
    q_eff = q_all * tln_g[None, :]                                      # [17,H]
    sq = q_eff.sum(axis=1)                                              # [17]
    w_bar = W_proj.mean(axis=1)                                         # [TOKD]
    Q_hat = W_proj @ q_eff.T - w_bar[:, None] * sq[None, :]             # [TOKD,17]
    # pre-rstd per-query score offset from the projection bias. (The LN-bias
    # term tln_b.q is a post-rstd per-query constant — a uniform softmax
    # rescale per query — and cancels in the context LN, so it's dropped.)
    bq = (b_proj @ q_eff.T) - float(b_proj.mean()) * sq

    W_aug = np.concatenate([W_proj, Q_hat], axis=1)                 # [TOKD,529]
    w_host = W_aug.reshape(KC, P, WTOT).transpose(1, 0, 2)          # [128,KC,529]
    bprow_host = np.concatenate([b_proj, bq]).reshape(1, WTOT)

    cg_host = np.ones((NQ, H), np.float32)
    cb_host = np.zeros((NQ, H), np.float32)
    cg_host[0] = gln_g
    cb_host[0] = gln_b
    cg_host[1:] = cln_g
    cb_host[1:] = cln_b

    bf16 = ml_dtypes.bfloat16

    skip = set()
    if np.all(tln_g == 1) and np.all(tln_b == 0):
        skip.add("tln")
    if np.all(cg_host == 1) and np.all(cb_host == 0):
        skip.add("gcln")
    if np.all(fln_g == 1) and np.all(fln_b == 0):
        skip.add("fln")
    if np.all(b_proj == 0) and np.all(bq == 0):
        skip.add("bp")
    if np.all(b_ff1 == 0):
        skip.add("bf1")
    if np.all(b_ff2 == 0):
        skip.add("bf2")

    nc = build_nc(S_c, b_out_s, frozenset(skip))

    NT = S_c // P
    TT = B_LOCAL * NT

    shared = dict(
        wp=w_host.reshape(P, KC * WTOT).astype(bf16),
        bprow=bprow_host.astype(bf16),
        tg=tln_g[None, :], tb=tln_b[None, :],
        cg=cg_host, cb=cb_host,
        fg=fln_g[None, :], fb=fln_b[None, :],
        w1=W_ff1.reshape(8, P, H).transpose(1, 0, 2).reshape(P, 8 * H).astype(bf16),
        bf1=b_ff1[None, :].astype(bf16),
        w2=W_ff2.reshape(4, P, H).transpose(1, 0, 2).reshape(P, 4 * H).astype(bf16),
        bf2=b_ff2[None, :].astype(bf16),
        wo=wo_host[None, :],
    )

    in_maps = []
    for c in range(N_CORES):
        m = dict(shared)
        bs = slice(c * B_LOCAL, (c + 1) * B_LOCAL)
        tsc = ts_c[bs].reshape(TT, P, KC, P)          # [tile, s, c, p]
        m["ts"] = np.ascontiguousarray(
            tsc.transpose(0, 3, 2, 1)).reshape(TT * P, TOKD).astype(bf16)
        m["pb"] = np.ascontiguousarray(
            padbias[bs].reshape(TT, P).T).astype(np.float32)
        in_maps.append(m)

    trace = os.environ.get("KERNEL_TRACE", "0") == "1"
    kw = {}
    if trace:
        kw = dict(trace=True, tmpdir=os.environ.get("KERNEL_TRACE_DIR") or None)
    res = run_bass_kernel_spmd(nc, in_maps, core_ids=list(range(N_CORES)), **kw)
    global LAST_RESULT
    LAST_RESULT = res
    outs = [res.results[c]["out"].T for c in range(N_CORES)]
    return np.concatenate(outs, axis=0).astype(np.float32)


if __name__ == "__main__":
    pass
